# revision 2
# baseline (speedup 1.0000x reference)
"""NonLocalBlock (embedded-gaussian-less, dot-product attention) TRN2 kernel.

Problem: x[16,256,64,64]; theta/phi/g = 1x1 conv to 128 ch; f = theta^T phi / HW;
y = f @ g^T (per batch); out conv back to 256 ch; BN(inference); residual add.

Sharding: data-parallel over batch. 8 cores x 2 batches each. No collectives.

Per-batch on-device schedule (per core, fully unrolled, 2 batches):
  theta/phi/g : [IC=128, HW=4096] = W^T-chunked matmuls over C=256 (2 k-chunks),
                bias fused into the PSUM->SBUF copy on the scalar engine.
  gT          : 32 PE-transposes of g's [128,128] column blocks (y-matmul needs
                the HW dim of g on partitions).
  main loop   : for each of 8 i-chunks (512 cols of f):
                  for j in 32: fT_j = phi_j^T theta_i (PSUM), copy to SBUF
                               (alternating scalar/vector engines),
                               y_i += gT_j^T fT_j  (PSUM accumulation over j).
  out conv    : w_out' y + (residual x + folded BN/bias) via one DVE
                scalar_tensor_tensor per [128,512] tile, DMA straight out.

All matmuls use float32r (full-rate fp32 mode, 4x faster than plain fp32 on
the PE; N=512 >= 256 so the full-rate condition holds). BN scale/shift and
b_out are folded into w_out / a per-channel bias on the host; 1/HW is folded
into w_theta/b_theta.
"""

import numpy as np

B, C, H, W = 16, 256, 64, 64
HW = H * W          # 4096
IC = C // 2         # 128
NCORES = 8
BPC = B // NCORES   # batches per core = 2
NI = HW // 512      # 8 i-chunks of 512
NJ = HW // 128      # 32 j-chunks of 128
BN_EPS = 1e-5

_CACHE = {}


def _build_program():
    import concourse.bass as bass
    import concourse.mybir as mybir
    from concourse import tile, bacc
    from concourse.masks import make_identity

    dt = mybir.dt
    f32 = dt.float32
    f32r = dt.float32r
    AF = mybir.ActivationFunctionType
    ALU = mybir.AluOpType

    nc = bacc.Bacc(trn_type="TRN2", target_bir_lowering=False, debug=False)

    # ---- DRAM I/O ----
    x_d = nc.dram_tensor("x", [BPC, C, HW], f32, kind="ExternalInput").ap()
    wth_d = nc.dram_tensor("wth", [C, IC], f32, kind="ExternalInput").ap()   # w_theta^T/HW
    wph_d = nc.dram_tensor("wph", [C, IC], f32, kind="ExternalInput").ap()   # w_phi^T
    wg_d = nc.dram_tensor("wg", [C, IC], f32, kind="ExternalInput").ap()     # w_g^T
    wout_d = nc.dram_tensor("wout", [IC, C], f32, kind="ExternalInput").ap() # (s*w_out)^T
    bth_d = nc.dram_tensor("bth", [IC, 1], f32, kind="ExternalInput").ap()
    bph_d = nc.dram_tensor("bph", [IC, 1], f32, kind="ExternalInput").ap()
    bg_d = nc.dram_tensor("bg", [IC, 1], f32, kind="ExternalInput").ap()
    bout_d = nc.dram_tensor("bout", [C, 1], f32, kind="ExternalInput").ap()
    out_d = nc.dram_tensor("out", [BPC, C, HW], f32, kind="ExternalOutput").ap()

    with tile.TileContext(nc) as tc:
        with (
            tc.tile_pool(name="const", bufs=1) as cpool,
            tc.tile_pool(name="xin", bufs=2) as xpool,
            tc.tile_pool(name="big", bufs=1) as bigpool,
            tc.tile_pool(name="ft", bufs=4) as ftpool,
            tc.tile_pool(name="ot", bufs=4) as otpool,
            tc.tile_pool(name="ps", bufs=5, space="PSUM") as pspool,
            tc.tile_pool(name="acc", bufs=2, space="PSUM") as accpool,
        ):
            # ---- constants ----
            wth_sb = cpool.tile([128, 2, IC], f32r, name="wth_sb")
            wph_sb = cpool.tile([128, 2, IC], f32r, name="wph_sb")
            wg_sb = cpool.tile([128, 2, IC], f32r, name="wg_sb")
            wout_sb = cpool.tile([128, C], f32r, name="wout_sb")
            bth_sb = cpool.tile([128, 1], f32, name="bth_sb")
            bph_sb = cpool.tile([128, 1], f32, name="bph_sb")
            bg_sb = cpool.tile([128, 1], f32, name="bg_sb")
            bout_sb = cpool.tile([128, 2], f32, name="bout_sb")
            ident = cpool.tile([128, 128], f32, name="ident")

            for c in range(2):
                nc.sync.dma_start(wth_sb[:, c, :], wth_d[c * 128:(c + 1) * 128, :].bitcast(f32r))
                nc.sync.dma_start(wph_sb[:, c, :], wph_d[c * 128:(c + 1) * 128, :].bitcast(f32r))
                nc.sync.dma_start(wg_sb[:, c, :], wg_d[c * 128:(c + 1) * 128, :].bitcast(f32r))
                nc.sync.dma_start(bout_sb[:, c:c + 1], bout_d[c * 128:(c + 1) * 128, :])
            nc.sync.dma_start(wout_sb[:], wout_d[:].bitcast(f32r))
            nc.sync.dma_start(bth_sb[:], bth_d[:])
            nc.sync.dma_start(bph_sb[:], bph_d[:])
            nc.sync.dma_start(bg_sb[:], bg_d[:])
            make_identity(nc, ident[:])

            for b in range(BPC):
                # ---- load x ----
                x_sb = xpool.tile([128, 2, HW], f32r, name="x_sb", tag="x")
                for c in range(2):
                    nc.sync.dma_start(x_sb[:, c, :], x_d[b, c * 128:(c + 1) * 128, :].bitcast(f32r))

                # ---- theta/phi/g convs ----
                th_sb = bigpool.tile([128, HW], f32r, name="th_sb", tag="th")
                ph_sb = bigpool.tile([128, HW], f32r, name="ph_sb", tag="ph")
                g_sb = bigpool.tile([128, HW], f32, name="g_sb", tag="g")
                for (w_sb, b_sb, o_sb) in (
                    (wth_sb, bth_sb, th_sb),
                    (wph_sb, bph_sb, ph_sb),
                    (wg_sb, bg_sb, g_sb),
                ):
                    for i in range(NI):
                        isl = slice(i * 512, (i + 1) * 512)
                        ps = pspool.tile([128, 512], f32, name="ps", tag="ps")
                        for c in range(2):
                            nc.tensor.matmul(
                                ps[:],
                                w_sb[:, c, :],
                                x_sb[:, c, isl],
                                start=(c == 0),
                                stop=(c == 1),
                            )
                        nc.scalar.activation(o_sb[:, isl], ps[:], AF.Identity,
                                             bias=b_sb[:], scale=1.0)

                # ---- gT via PE transpose ----
                gt_sb = bigpool.tile([128, HW], f32r, name="gt_sb", tag="gt")
                for j in range(NJ):
                    jsl = slice(j * 128, (j + 1) * 128)
                    tp = pspool.tile([128, 128], f32, name="tp", tag="ps")
                    nc.tensor.transpose(tp[:], g_sb[:, jsl], ident[:])
                    nc.vector.tensor_copy(gt_sb[:, jsl], tp[:])

                # ---- main attention loop ----
                y_sb = bigpool.tile([128, HW], f32r, name="y_sb", tag="y")
                for i in range(NI):
                    isl = slice(i * 512, (i + 1) * 512)
                    acc = accpool.tile([128, 512], f32, name="acc", tag="acc")
                    fts = []
                    pend = []  # deferred y-matmuls for SW pipelining (depth 2)
                    for j in range(NJ):
                        jsl = slice(j * 128, (j + 1) * 128)
                        psf = pspool.tile([128, 512], f32, name="psf", tag="ps")
                        nc.tensor.matmul(
                            psf[:],
                            ph_sb[:, jsl],
                            th_sb[:, isl],
                            start=True, stop=True,
                        )
                        ft = ftpool.tile([128, 512], f32r, name="ft", tag="ft")
                        if j % 2 == 0:
                            nc.scalar.copy(ft[:], psf[:])
                        else:
                            nc.vector.tensor_copy(ft[:], psf[:])
                        pend.append((j, ft))
                        if len(pend) > 2:
                            jj, ftj = pend.pop(0)
                            nc.tensor.matmul(
                                acc[:], gt_sb[:, jj * 128:(jj + 1) * 128],
                                ftj[:],
                                start=(jj == 0), stop=(jj == NJ - 1),
                            )
                    for (jj, ftj) in pend:
                        nc.tensor.matmul(
                            acc[:], gt_sb[:, jj * 128:(jj + 1) * 128],
                            ftj[:],
                            start=(jj == 0), stop=(jj == NJ - 1),
                        )
                    nc.vector.tensor_copy(y_sb[:, isl], acc[:])

                # ---- out conv + bias + residual, DMA out ----
                for i in range(NI):
                    isl = slice(i * 512, (i + 1) * 512)
                    for o in range(2):
                        ps2 = pspool.tile([128, 512], f32, name="ps2", tag="ps")
                        nc.tensor.matmul(
                            ps2[:],
                            wout_sb[:, o * 128:(o + 1) * 128],
                            y_sb[:, isl],
                            start=True, stop=True,
                        )
                        ot = otpool.tile([128, 512], f32, name="ot", tag="ot")
                        nc.vector.scalar_tensor_tensor(
                            ot[:], ps2[:], bout_sb[:, o:o + 1], x_sb[:, o, isl].bitcast(f32),
                            op0=ALU.add, op1=ALU.add,
                        )
                        nc.sync.dma_start(out_d[b, o * 128:(o + 1) * 128, isl], ot[:])

    nc.compile()
    return nc


def _get_program():
    if "nc" not in _CACHE:
        _CACHE["nc"] = _build_program()
    return _CACHE["nc"]


def kernel(x, w_theta, b_theta, w_phi, b_phi, w_g, b_g, w_out, b_out,
           bn_gamma, bn_beta, bn_mean, bn_var):
    from concourse.bass_utils import run_bass_kernel_spmd

    x = np.asarray(x, dtype=np.float32)
    w_theta = np.asarray(w_theta, np.float32); b_theta = np.asarray(b_theta, np.float32)
    w_phi = np.asarray(w_phi, np.float32); b_phi = np.asarray(b_phi, np.float32)
    w_g = np.asarray(w_g, np.float32); b_g = np.asarray(b_g, np.float32)
    w_out = np.asarray(w_out, np.float32); b_out = np.asarray(b_out, np.float32)
    bn_gamma = np.asarray(bn_gamma, np.float32); bn_beta = np.asarray(bn_beta, np.float32)
    bn_mean = np.asarray(bn_mean, np.float32); bn_var = np.asarray(bn_var, np.float32)

    # host-side folding
    s = bn_gamma / np.sqrt(bn_var + BN_EPS)              # BN scale
    wout_f = (s[:, None] * w_out)                        # [C, IC]
    bout_f = s * b_out + bn_beta - bn_mean * s           # [C]

    wth = np.ascontiguousarray(w_theta.T / HW)           # [C, IC], 1/HW folded
    bth = (b_theta / HW).reshape(IC, 1)
    wph = np.ascontiguousarray(w_phi.T)
    bph = b_phi.reshape(IC, 1)
    wg = np.ascontiguousarray(w_g.T)
    bg = b_g.reshape(IC, 1)
    wout = np.ascontiguousarray(wout_f.T)                # [IC, C]
    bout = bout_f.reshape(C, 1).astype(np.float32)

    xr = np.ascontiguousarray(x.reshape(B, C, HW))

    nc = _get_program()
    in_maps = []
    for core in range(NCORES):
        in_maps.append({
            "x": xr[core * BPC:(core + 1) * BPC],
            "wth": wth, "wph": wph, "wg": wg, "wout": wout,
            "bth": bth, "bph": bph, "bg": bg, "bout": bout,
        })
    res = run_bass_kernel_spmd(nc, in_maps, core_ids=list(range(NCORES)))
    out = np.concatenate([res.results[c]["out"] for c in range(NCORES)], axis=0)
    return out.reshape(B, C, H, W)


# revision 9
# speedup vs baseline: 1.1459x; 1.1459x over previous
"""NonLocalBlock (embedded-gaussian-less, dot-product attention) TRN2 kernel.

Problem: x[16,256,64,64]; theta/phi/g = 1x1 conv to 128 ch; f = theta^T phi / HW;
y = f @ g^T (per batch); out conv back to 256 ch; BN(inference); residual add.

Sharding: data-parallel over batch. 8 cores x 2 batches each. No collectives.

Per-batch on-device schedule (per core, fully unrolled, 2 batches):
  theta/phi/g : [IC=128, HW=4096] = W^T-chunked matmuls over C=256 (2 k-chunks),
                bias fused into the PSUM->SBUF copy on the scalar engine.
  gT          : 32 PE-transposes of g's [128,128] column blocks (y-matmul needs
                the HW dim of g on partitions).
  main loop   : for each of 8 i-chunks (512 cols of f):
                  for j in 32: fT_j = phi_j^T theta_i (PSUM), copy to SBUF
                               (alternating scalar/vector engines),
                               y_i += gT_j^T fT_j  (PSUM accumulation over j).
  out conv    : w_out' y + (residual x + folded BN/bias) via one DVE
                scalar_tensor_tensor per [128,512] tile, DMA straight out.

All matmuls use float32r (full-rate fp32 mode, 4x faster than plain fp32 on
the PE; N=512 >= 256 so the full-rate condition holds). BN scale/shift and
b_out are folded into w_out / a per-channel bias on the host; 1/HW is folded
into w_theta/b_theta.
"""

import numpy as np

B, C, H, W = 16, 256, 64, 64
HW = H * W          # 4096
IC = C // 2         # 128
NCORES = 8
BPC = B // NCORES   # batches per core = 2
NI = HW // 512      # 8 i-chunks of 512
NJ = HW // 128      # 32 j-chunks of 128
BN_EPS = 1e-5

_CACHE = {}


def _build_program():
    import concourse.bass as bass
    import concourse.mybir as mybir
    from concourse import tile, bacc
    from concourse.masks import make_identity

    dt = mybir.dt
    f32 = dt.float32
    f32r = dt.float32r
    AF = mybir.ActivationFunctionType
    ALU = mybir.AluOpType

    nc = bacc.Bacc(trn_type="TRN2", target_bir_lowering=False, debug=False)

    # ---- DRAM I/O ----
    x_d = nc.dram_tensor("x", [BPC, C, HW], f32, kind="ExternalInput").ap()
    # wcat columns: [wth_c0|wth_c1|wph_c0|wph_c1|wg_c0|wg_c1|woutT] = 6*128+256 = 1024
    wcat_d = nc.dram_tensor("wcat", [128, 1024], f32, kind="ExternalInput").ap()
    # bcat columns: [bth, bph, bg, bout_c0, bout_c1]
    bcat_d = nc.dram_tensor("bcat", [128, 5], f32, kind="ExternalInput").ap()
    out_d = nc.dram_tensor("out", [BPC, C, HW], f32, kind="ExternalOutput").ap()

    with tile.TileContext(nc) as tc:
        with (
            tc.tile_pool(name="const", bufs=1) as cpool,
            tc.tile_pool(name="xin", bufs=2) as xpool,
            tc.tile_pool(name="big", bufs=1) as bigpool,
            tc.tile_pool(name="ft", bufs=4) as ftpool,
            tc.tile_pool(name="ot", bufs=4) as otpool,
            tc.tile_pool(name="ps", bufs=6, space="PSUM") as pspool,
            tc.tile_pool(name="acc", bufs=2, space="PSUM") as accpool,
        ):
            # ---- constants + x loads (i0 first so compute starts ASAP) ----
            wcat_sb = cpool.tile([128, 1024], f32r, name="wcat_sb")
            bcat_sb = cpool.tile([128, 5], f32, name="bcat_sb")
            ident = cpool.tile([128, 128], f32, name="ident")

            x_tiles = [xpool.tile([128, 2, HW], f32r, name=f"x_sb{b}", tag="x")
                       for b in range(BPC)]
            # batch 0, chunk i0 first; then weights; then the rest
            nc.sync.dma_start(wcat_sb[:], wcat_d[:].bitcast(f32r))
            for c in range(2):
                nc.sync.dma_start(x_tiles[0][:, c, 0:512],
                                  x_d[0, c * 128:(c + 1) * 128, 0:512].bitcast(f32r))
            nc.sync.dma_start(bcat_sb[:], bcat_d[:])
            make_identity(nc, ident[:])
            for b in range(BPC):
                for (lo, hi) in ((512, 1536), (1536, 2560), (2560, 3584), (3584, 4096)):
                    for c in range(2):
                        nc.sync.dma_start(x_tiles[b][:, c, lo:hi],
                                          x_d[b, c * 128:(c + 1) * 128, lo:hi].bitcast(f32r))
                if b > 0:
                    for c in range(2):
                        nc.sync.dma_start(x_tiles[b][:, c, 0:512],
                                          x_d[b, c * 128:(c + 1) * 128, 0:512].bitcast(f32r))

            wth_sb = wcat_sb[:, 0:256].rearrange("p (k m) -> p k m", k=2)
            wph_sb = wcat_sb[:, 256:512].rearrange("p (k m) -> p k m", k=2)
            wg_sb = wcat_sb[:, 512:768].rearrange("p (k m) -> p k m", k=2)
            wout_sb = wcat_sb[:, 768:1024]
            bth_sb = bcat_sb[:, 0:1]
            bph_sb = bcat_sb[:, 1:2]
            bg_sb = bcat_sb[:, 2:3]
            bout_sb = bcat_sb[:, 3:5]

            for b in range(BPC):
                x_sb = x_tiles[b]

                # ---- theta/phi/g convs ----
                th_sb = bigpool.tile([128, HW], f32r, name="th_sb", tag="th")
                ph_sb = bigpool.tile([128, HW], f32r, name="ph_sb", tag="ph")
                g_sb = bigpool.tile([128, HW], f32, name="g_sb", tag="g")
                for i in range(NI):
                    isl = slice(i * 512, (i + 1) * 512)
                    for k, (w_sb, b_sb, o_sb) in enumerate((
                        (wth_sb, bth_sb, th_sb),
                        (wph_sb, bph_sb, ph_sb),
                        (wg_sb, bg_sb, g_sb),
                    )):
                        ps = pspool.tile([128, 512], f32, name="ps", tag="ps")
                        for c in range(2):
                            nc.tensor.matmul(
                                ps[:],
                                w_sb[:, c, :],
                                x_sb[:, c, isl],
                                start=(c == 0),
                                stop=(c == 1),
                            )
                        if (i + k) % 2 == 0:
                            nc.scalar.activation(o_sb[:, isl], ps[:], AF.Identity,
                                                 bias=b_sb[:], scale=1.0)
                        else:
                            nc.vector.tensor_scalar_add(o_sb[:, isl], ps[:], b_sb[:])

                # gT tiles produced inside the first main-loop chunk (below)
                gt_sb = bigpool.tile([128, HW], f32r, name="gt_sb", tag="gt")

                # ---- main attention loop (out-conv folded in, deferred) ----
                y_sb = bigpool.tile([128, HW], f32r, name="y_sb", tag="y")

                def emit_outconv(i):
                    isl2 = slice(i * 512, (i + 1) * 512)
                    for o in range(2):
                        ps2 = pspool.tile([128, 512], f32, name="ps2", tag="ps")
                        nc.tensor.matmul(
                            ps2[:],
                            wout_sb[:, o * 128:(o + 1) * 128],
                            y_sb[:, isl2],
                            start=True, stop=True,
                        )
                        ot = otpool.tile([128, 512], f32, name="ot", tag="ot")
                        nc.vector.scalar_tensor_tensor(
                            ot[:], ps2[:], bout_sb[:, o:o + 1], x_sb[:, o, isl2].bitcast(f32),
                            op0=ALU.add, op1=ALU.add,
                        )
                        nc.sync.dma_start(out_d[b, o * 128:(o + 1) * 128, isl2], ot[:])

                for i in range(NI):
                    isl = slice(i * 512, (i + 1) * 512)
                    acc = accpool.tile([128, 512], f32, name="acc", tag="acc")
                    pend = []  # deferred y-matmuls for SW pipelining (depth 3)
                    for j in range(NJ):
                        jsl = slice(j * 128, (j + 1) * 128)
                        psf = pspool.tile([128, 512], f32, name="psf", tag="ps")
                        nc.tensor.matmul(
                            psf[:],
                            ph_sb[:, jsl],
                            th_sb[:, isl],
                            start=True, stop=True,
                        )
                        ft = ftpool.tile([128, 512], f32r, name="ft", tag="ft")
                        if i == 0:
                            # produce gT_j here so the transpose phase overlaps
                            tp = pspool.tile([128, 128], f32, name="tp", tag="ps")
                            nc.tensor.transpose(tp[:], g_sb[:, jsl], ident[:])
                            if j % 2 == 0:
                                nc.scalar.copy(ft[:], psf[:])
                                nc.vector.tensor_copy(gt_sb[:, jsl], tp[:])
                            else:
                                nc.vector.tensor_copy(ft[:], psf[:])
                                nc.scalar.copy(gt_sb[:, jsl], tp[:])
                        else:
                            if j % 2 == 0:
                                nc.scalar.copy(ft[:], psf[:])
                            else:
                                nc.vector.tensor_copy(ft[:], psf[:])
                        pend.append((j, ft))
                        if len(pend) > 3:
                            jj, ftj = pend.pop(0)
                            nc.tensor.matmul(
                                acc[:], gt_sb[:, jj * 128:(jj + 1) * 128],
                                ftj[:],
                                start=(jj == 0), stop=(jj == NJ - 1),
                            )
                        # out-conv of chunk i-1 once chunk i is a few fT mms in
                        if j == 4 and i > 0:
                            emit_outconv(i - 1)
                    for (jj, ftj) in pend:
                        nc.tensor.matmul(
                            acc[:], gt_sb[:, jj * 128:(jj + 1) * 128],
                            ftj[:],
                            start=(jj == 0), stop=(jj == NJ - 1),
                        )
                    if i == NI - 1:
                        nc.scalar.copy(y_sb[:, i * 512:i * 512 + 256], acc[:, 0:256])
                        nc.vector.tensor_copy(y_sb[:, i * 512 + 256:(i + 1) * 512], acc[:, 256:512])
                    else:
                        nc.scalar.copy(y_sb[:, isl], acc[:])
                emit_outconv(NI - 1)

    nc.compile()
    return nc


def _get_program():
    if "nc" not in _CACHE:
        _CACHE["nc"] = _build_program()
    return _CACHE["nc"]


def _make_in_maps(inputs):
    """Host-side prep: fold BN/bias/scale, pack weights, slice batches."""
    x = np.asarray(inputs["x"], dtype=np.float32)
    w_theta = inputs["w_theta"]; b_theta = inputs["b_theta"]
    w_phi = inputs["w_phi"]; b_phi = inputs["b_phi"]
    w_g = inputs["w_g"]; b_g = inputs["b_g"]
    w_out = inputs["w_out"]; b_out = inputs["b_out"]
    bn_gamma = inputs["bn_gamma"]; bn_beta = inputs["bn_beta"]
    bn_mean = inputs["bn_mean"]; bn_var = inputs["bn_var"]
    w_theta = np.asarray(w_theta, np.float32); b_theta = np.asarray(b_theta, np.float32)
    w_phi = np.asarray(w_phi, np.float32); b_phi = np.asarray(b_phi, np.float32)
    w_g = np.asarray(w_g, np.float32); b_g = np.asarray(b_g, np.float32)
    w_out = np.asarray(w_out, np.float32); b_out = np.asarray(b_out, np.float32)
    bn_gamma = np.asarray(bn_gamma, np.float32); bn_beta = np.asarray(bn_beta, np.float32)
    bn_mean = np.asarray(bn_mean, np.float32); bn_var = np.asarray(bn_var, np.float32)

    # host-side folding
    s = bn_gamma / np.sqrt(bn_var + BN_EPS)              # BN scale
    wout_f = (s[:, None] * w_out)                        # [C, IC]
    bout_f = s * b_out + bn_beta - bn_mean * s           # [C]

    wth = w_theta.T / HW                                 # [C, IC], 1/HW folded
    wph = w_phi.T
    wg = w_g.T
    wout = wout_f.T                                      # [IC, C]

    # wcat: [wth_c0|wth_c1|wph_c0|wph_c1|wg_c0|wg_c1|woutT] -> [128, 1024]
    wcat = np.concatenate(
        [wth[0:128], wth[128:256], wph[0:128], wph[128:256],
         wg[0:128], wg[128:256], wout], axis=1).astype(np.float32)
    wcat = np.ascontiguousarray(wcat)
    # bcat: [bth, bph, bg, bout_c0, bout_c1] -> [128, 5]
    bcat = np.stack(
        [b_theta / HW, b_phi, b_g, bout_f[0:128], bout_f[128:256]],
        axis=1).astype(np.float32)
    bcat = np.ascontiguousarray(bcat)

    xr = np.ascontiguousarray(x.reshape(B, C, HW))

    in_maps = []
    for core in range(NCORES):
        in_maps.append({
            "x": xr[core * BPC:(core + 1) * BPC],
            "wcat": wcat, "bcat": bcat,
        })
    return in_maps


def kernel(x, w_theta, b_theta, w_phi, b_phi, w_g, b_g, w_out, b_out,
           bn_gamma, bn_beta, bn_mean, bn_var):
    from concourse.bass_utils import run_bass_kernel_spmd

    in_maps = _make_in_maps(dict(
        x=x, w_theta=w_theta, b_theta=b_theta, w_phi=w_phi, b_phi=b_phi,
        w_g=w_g, b_g=b_g, w_out=w_out, b_out=b_out, bn_gamma=bn_gamma,
        bn_beta=bn_beta, bn_mean=bn_mean, bn_var=bn_var))
    nc = _get_program()
    res = run_bass_kernel_spmd(nc, in_maps, core_ids=list(range(NCORES)))
    out = np.concatenate([res.results[c]["out"] for c in range(NCORES)], axis=0)
    return out.reshape(B, C, H, W)


# revision 10
# speedup vs baseline: 7.2072x; 6.2898x over previous
"""NonLocalBlock (embedded-gaussian-less, dot-product attention) TRN2 kernel.

Problem: x[16,256,64,64]; theta/phi/g = 1x1 conv to 128 ch; f = theta^T phi / HW;
y = f @ g^T (per batch); out conv back to 256 ch; BN(inference); residual add.

Sharding: data-parallel over batch. 8 cores x 2 batches each. No collectives.

Per-batch on-device schedule (per core, fully unrolled, 2 batches):
  theta/phi/g : [IC=128, HW=4096] = W^T-chunked matmuls over C=256 (2 k-chunks),
                bias fused into the PSUM->SBUF copy on the scalar engine.
  gT          : 32 PE-transposes of g's [128,128] column blocks (y-matmul needs
                the HW dim of g on partitions).
  main loop   : for each of 8 i-chunks (512 cols of f):
                  for j in 32: fT_j = phi_j^T theta_i (PSUM), copy to SBUF
                               (alternating scalar/vector engines),
                               y_i += gT_j^T fT_j  (PSUM accumulation over j).
  out conv    : w_out' y + (residual x + folded BN/bias) via one DVE
                scalar_tensor_tensor per [128,512] tile, DMA straight out.

All matmuls use float32r (full-rate fp32 mode, 4x faster than plain fp32 on
the PE; N=512 >= 256 so the full-rate condition holds). BN scale/shift and
b_out are folded into w_out / a per-channel bias on the host; 1/HW is folded
into w_theta/b_theta.
"""

import numpy as np

B, C, H, W = 16, 256, 64, 64
HW = H * W          # 4096
IC = C // 2         # 128
NCORES = 8
BPC = B // NCORES   # batches per core = 2
NI = HW // 512      # 8 i-chunks of 512
NJ = HW // 128      # 32 j-chunks of 128
BN_EPS = 1e-5

_CACHE = {}


def _build_program(loop_n=1):
    import concourse.bass as bass
    import concourse.mybir as mybir
    from concourse import tile, bacc
    from concourse.masks import make_identity
    from contextlib import ExitStack

    dt = mybir.dt
    f32 = dt.float32
    f32r = dt.float32r
    AF = mybir.ActivationFunctionType
    ALU = mybir.AluOpType

    nc = bacc.Bacc(trn_type="TRN2", target_bir_lowering=False, debug=False)

    # ---- DRAM I/O ----
    x_d = nc.dram_tensor("x", [BPC, C, HW], f32, kind="ExternalInput").ap()
    # wcat columns: [wth_c0|wth_c1|wph_c0|wph_c1|wg_c0|wg_c1|woutT] = 6*128+256 = 1024
    wcat_d = nc.dram_tensor("wcat", [128, 1024], f32, kind="ExternalInput").ap()
    # bcat columns: [bth, bph, bg, bout_c0, bout_c1]
    bcat_d = nc.dram_tensor("bcat", [128, 5], f32, kind="ExternalInput").ap()
    out_d = nc.dram_tensor("out", [BPC, C, HW], f32, kind="ExternalOutput").ap()

    with tile.TileContext(nc) as tc:
        with (
            tc.tile_pool(name="const", bufs=1) as cpool,
            tc.tile_pool(name="xin", bufs=2) as xpool,
            tc.tile_pool(name="big", bufs=1) as bigpool,
            tc.tile_pool(name="ft", bufs=4) as ftpool,
            tc.tile_pool(name="ot", bufs=4) as otpool,
            tc.tile_pool(name="ps", bufs=6, space="PSUM") as pspool,
            tc.tile_pool(name="acc", bufs=2, space="PSUM") as accpool,
        ):
            # ---- constants + x loads (i0 first so compute starts ASAP) ----
            wcat_sb = cpool.tile([128, 1024], f32r, name="wcat_sb")
            bcat_sb = cpool.tile([128, 5], f32, name="bcat_sb")
            ident = cpool.tile([128, 128], f32, name="ident")

            loop_ctx = ExitStack()
            if loop_n > 1:
                loop_ctx.enter_context(tc.For_i(0, loop_n, 1))

            x_tiles = [xpool.tile([128, 2, HW], f32r, name=f"x_sb{b}", tag="x")
                       for b in range(BPC)]
            # batch 0, chunk i0 first; then weights; then the rest
            nc.sync.dma_start(wcat_sb[:], wcat_d[:].bitcast(f32r))
            for c in range(2):
                nc.sync.dma_start(x_tiles[0][:, c, 0:512],
                                  x_d[0, c * 128:(c + 1) * 128, 0:512].bitcast(f32r))
            nc.sync.dma_start(bcat_sb[:], bcat_d[:])
            make_identity(nc, ident[:])
            for b in range(BPC):
                for (lo, hi) in ((512, 1536), (1536, 2560), (2560, 3584), (3584, 4096)):
                    for c in range(2):
                        nc.sync.dma_start(x_tiles[b][:, c, lo:hi],
                                          x_d[b, c * 128:(c + 1) * 128, lo:hi].bitcast(f32r))
                if b > 0:
                    for c in range(2):
                        nc.sync.dma_start(x_tiles[b][:, c, 0:512],
                                          x_d[b, c * 128:(c + 1) * 128, 0:512].bitcast(f32r))

            wth_sb = wcat_sb[:, 0:256].rearrange("p (k m) -> p k m", k=2)
            wph_sb = wcat_sb[:, 256:512].rearrange("p (k m) -> p k m", k=2)
            wg_sb = wcat_sb[:, 512:768].rearrange("p (k m) -> p k m", k=2)
            wout_sb = wcat_sb[:, 768:1024]
            bth_sb = bcat_sb[:, 0:1]
            bph_sb = bcat_sb[:, 1:2]
            bg_sb = bcat_sb[:, 2:3]
            bout_sb = bcat_sb[:, 3:5]

            for b in range(BPC):
                x_sb = x_tiles[b]

                # ---- theta/phi/g convs ----
                th_sb = bigpool.tile([128, HW], f32r, name="th_sb", tag="th")
                ph_sb = bigpool.tile([128, HW], f32r, name="ph_sb", tag="ph")
                g_sb = bigpool.tile([128, HW], f32, name="g_sb", tag="g")
                for i in range(NI):
                    isl = slice(i * 512, (i + 1) * 512)
                    for k, (w_sb, b_sb, o_sb) in enumerate((
                        (wth_sb, bth_sb, th_sb),
                        (wph_sb, bph_sb, ph_sb),
                        (wg_sb, bg_sb, g_sb),
                    )):
                        ps = pspool.tile([128, 512], f32, name="ps", tag="ps")
                        for c in range(2):
                            nc.tensor.matmul(
                                ps[:],
                                w_sb[:, c, :],
                                x_sb[:, c, isl],
                                start=(c == 0),
                                stop=(c == 1),
                            )
                        if (i + k) % 2 == 0:
                            nc.scalar.activation(o_sb[:, isl], ps[:], AF.Identity,
                                                 bias=b_sb[:], scale=1.0)
                        else:
                            nc.vector.tensor_scalar_add(o_sb[:, isl], ps[:], b_sb[:])

                # gT tiles produced inside the first main-loop chunk (below)
                gt_sb = bigpool.tile([128, HW], f32r, name="gt_sb", tag="gt")

                # ---- main attention loop (out-conv folded in, deferred) ----
                y_sb = bigpool.tile([128, HW], f32r, name="y_sb", tag="y")

                def emit_outconv(i):
                    isl2 = slice(i * 512, (i + 1) * 512)
                    for o in range(2):
                        ps2 = pspool.tile([128, 512], f32, name="ps2", tag="ps")
                        nc.tensor.matmul(
                            ps2[:],
                            wout_sb[:, o * 128:(o + 1) * 128],
                            y_sb[:, isl2],
                            start=True, stop=True,
                        )
                        ot = otpool.tile([128, 512], f32, name="ot", tag="ot")
                        nc.vector.scalar_tensor_tensor(
                            ot[:], ps2[:], bout_sb[:, o:o + 1], x_sb[:, o, isl2].bitcast(f32),
                            op0=ALU.add, op1=ALU.add,
                        )
                        nc.sync.dma_start(out_d[b, o * 128:(o + 1) * 128, isl2], ot[:])

                for i in range(NI):
                    isl = slice(i * 512, (i + 1) * 512)
                    acc = accpool.tile([128, 512], f32, name="acc", tag="acc")
                    pend = []  # deferred y-matmuls for SW pipelining (depth 3)
                    for j in range(NJ):
                        jsl = slice(j * 128, (j + 1) * 128)
                        psf = pspool.tile([128, 512], f32, name="psf", tag="ps")
                        nc.tensor.matmul(
                            psf[:],
                            ph_sb[:, jsl],
                            th_sb[:, isl],
                            start=True, stop=True,
                        )
                        ft = ftpool.tile([128, 512], f32r, name="ft", tag="ft")
                        if i == 0:
                            # produce gT_j here so the transpose phase overlaps
                            tp = pspool.tile([128, 128], f32, name="tp", tag="ps")
                            nc.tensor.transpose(tp[:], g_sb[:, jsl], ident[:])
                            if j % 2 == 0:
                                nc.scalar.copy(ft[:], psf[:])
                                nc.vector.tensor_copy(gt_sb[:, jsl], tp[:])
                            else:
                                nc.vector.tensor_copy(ft[:], psf[:])
                                nc.scalar.copy(gt_sb[:, jsl], tp[:])
                        else:
                            if j % 2 == 0:
                                nc.scalar.copy(ft[:], psf[:])
                            else:
                                nc.vector.tensor_copy(ft[:], psf[:])
                        pend.append((j, ft))
                        if len(pend) > 3:
                            jj, ftj = pend.pop(0)
                            nc.tensor.matmul(
                                acc[:], gt_sb[:, jj * 128:(jj + 1) * 128],
                                ftj[:],
                                start=(jj == 0), stop=(jj == NJ - 1),
                            )
                        # out-conv of chunk i-1 once chunk i is a few fT mms in
                        if j == 4 and i > 0:
                            emit_outconv(i - 1)
                    for (jj, ftj) in pend:
                        nc.tensor.matmul(
                            acc[:], gt_sb[:, jj * 128:(jj + 1) * 128],
                            ftj[:],
                            start=(jj == 0), stop=(jj == NJ - 1),
                        )
                    if i == NI - 1:
                        nc.scalar.copy(y_sb[:, i * 512:i * 512 + 256], acc[:, 0:256])
                        nc.vector.tensor_copy(y_sb[:, i * 512 + 256:(i + 1) * 512], acc[:, 256:512])
                    else:
                        nc.scalar.copy(y_sb[:, isl], acc[:])
                emit_outconv(NI - 1)

            loop_ctx.close()

    nc.compile()
    return nc


def _get_program(loop_n=1):
    key = ("nc", loop_n)
    if key not in _CACHE:
        _CACHE[key] = _build_program(loop_n)
    return _CACHE[key]


def _make_in_maps(inputs):
    """Host-side prep: fold BN/bias/scale, pack weights, slice batches."""
    x = np.asarray(inputs["x"], dtype=np.float32)
    w_theta = inputs["w_theta"]; b_theta = inputs["b_theta"]
    w_phi = inputs["w_phi"]; b_phi = inputs["b_phi"]
    w_g = inputs["w_g"]; b_g = inputs["b_g"]
    w_out = inputs["w_out"]; b_out = inputs["b_out"]
    bn_gamma = inputs["bn_gamma"]; bn_beta = inputs["bn_beta"]
    bn_mean = inputs["bn_mean"]; bn_var = inputs["bn_var"]
    w_theta = np.asarray(w_theta, np.float32); b_theta = np.asarray(b_theta, np.float32)
    w_phi = np.asarray(w_phi, np.float32); b_phi = np.asarray(b_phi, np.float32)
    w_g = np.asarray(w_g, np.float32); b_g = np.asarray(b_g, np.float32)
    w_out = np.asarray(w_out, np.float32); b_out = np.asarray(b_out, np.float32)
    bn_gamma = np.asarray(bn_gamma, np.float32); bn_beta = np.asarray(bn_beta, np.float32)
    bn_mean = np.asarray(bn_mean, np.float32); bn_var = np.asarray(bn_var, np.float32)

    # host-side folding
    s = bn_gamma / np.sqrt(bn_var + BN_EPS)              # BN scale
    wout_f = (s[:, None] * w_out)                        # [C, IC]
    bout_f = s * b_out + bn_beta - bn_mean * s           # [C]

    wth = w_theta.T / HW                                 # [C, IC], 1/HW folded
    wph = w_phi.T
    wg = w_g.T
    wout = wout_f.T                                      # [IC, C]

    # wcat: [wth_c0|wth_c1|wph_c0|wph_c1|wg_c0|wg_c1|woutT] -> [128, 1024]
    wcat = np.concatenate(
        [wth[0:128], wth[128:256], wph[0:128], wph[128:256],
         wg[0:128], wg[128:256], wout], axis=1).astype(np.float32)
    wcat = np.ascontiguousarray(wcat)
    # bcat: [bth, bph, bg, bout_c0, bout_c1] -> [128, 5]
    bcat = np.stack(
        [b_theta / HW, b_phi, b_g, bout_f[0:128], bout_f[128:256]],
        axis=1).astype(np.float32)
    bcat = np.ascontiguousarray(bcat)

    xr = np.ascontiguousarray(x.reshape(B, C, HW))

    in_maps = []
    for core in range(NCORES):
        in_maps.append({
            "x": xr[core * BPC:(core + 1) * BPC],
            "wcat": wcat, "bcat": bcat,
        })
    return in_maps


def kernel(x, w_theta, b_theta, w_phi, b_phi, w_g, b_g, w_out, b_out,
           bn_gamma, bn_beta, bn_mean, bn_var):
    from concourse.bass_utils import run_bass_kernel_spmd

    in_maps = _make_in_maps(dict(
        x=x, w_theta=w_theta, b_theta=b_theta, w_phi=w_phi, b_phi=b_phi,
        w_g=w_g, b_g=b_g, w_out=w_out, b_out=b_out, bn_gamma=bn_gamma,
        bn_beta=bn_beta, bn_mean=bn_mean, bn_var=bn_var))
    nc = _get_program()
    res = run_bass_kernel_spmd(nc, in_maps, core_ids=list(range(NCORES)))
    out = np.concatenate([res.results[c]["out"] for c in range(NCORES)], axis=0)
    return out.reshape(B, C, H, W)


# revision 11
# speedup vs baseline: 7.3814x; 1.0242x over previous
"""NonLocalBlock (embedded-gaussian-less, dot-product attention) TRN2 kernel.

Problem: x[16,256,64,64]; theta/phi/g = 1x1 conv to 128 ch; f = theta^T phi / HW;
y = f @ g^T (per batch); out conv back to 256 ch; BN(inference); residual add.

Sharding: data-parallel over batch. 8 cores x 2 batches each. No collectives.

Per-batch on-device schedule (per core, fully unrolled, 2 batches):
  theta/phi/g : [IC=128, HW=4096] = W^T-chunked matmuls over C=256 (2 k-chunks),
                bias fused into the PSUM->SBUF copy on the scalar engine.
  gT          : 32 PE-transposes of g's [128,128] column blocks (y-matmul needs
                the HW dim of g on partitions).
  main loop   : for each of 8 i-chunks (512 cols of f):
                  for j in 32: fT_j = phi_j^T theta_i (PSUM), copy to SBUF
                               (alternating scalar/vector engines),
                               y_i += gT_j^T fT_j  (PSUM accumulation over j).
  out conv    : w_out' y + (residual x + folded BN/bias) via one DVE
                scalar_tensor_tensor per [128,512] tile, DMA straight out.

All matmuls use float32r (full-rate fp32 mode, 4x faster than plain fp32 on
the PE; N=512 >= 256 so the full-rate condition holds). BN scale/shift and
b_out are folded into w_out / a per-channel bias on the host; 1/HW is folded
into w_theta/b_theta.
"""

import numpy as np

B, C, H, W = 16, 256, 64, 64
HW = H * W          # 4096
IC = C // 2         # 128
NCORES = 8
BPC = B // NCORES   # batches per core = 2
NI = HW // 512      # 8 i-chunks of 512
NJ = HW // 128      # 32 j-chunks of 128
BN_EPS = 1e-5

_CACHE = {}


def _build_program(loop_n=1):
    import concourse.bass as bass
    import concourse.mybir as mybir
    from concourse import tile, bacc
    from concourse.masks import make_identity
    from contextlib import ExitStack

    dt = mybir.dt
    f32 = dt.float32
    f32r = dt.float32r
    AF = mybir.ActivationFunctionType
    ALU = mybir.AluOpType

    nc = bacc.Bacc(trn_type="TRN2", target_bir_lowering=False, debug=False)

    # ---- DRAM I/O ----
    x_d = nc.dram_tensor("x", [BPC, C, HW], f32, kind="ExternalInput").ap()
    # wcat columns: [wth_c0|wth_c1|wph_c0|wph_c1|wg_c0|wg_c1|woutT] = 6*128+256 = 1024
    wcat_d = nc.dram_tensor("wcat", [128, 1024], f32, kind="ExternalInput").ap()
    # bcat columns: [bth, bph, bg, bout_c0, bout_c1]
    bcat_d = nc.dram_tensor("bcat", [128, 5], f32, kind="ExternalInput").ap()
    out_d = nc.dram_tensor("out", [BPC, C, HW], f32, kind="ExternalOutput").ap()

    with tile.TileContext(nc) as tc:
        with (
            tc.tile_pool(name="const", bufs=1) as cpool,
            tc.tile_pool(name="xin", bufs=2) as xpool,
            tc.tile_pool(name="big", bufs=1) as bigpool,
            tc.tile_pool(name="ft", bufs=4) as ftpool,
            tc.tile_pool(name="ot", bufs=4) as otpool,
            tc.tile_pool(name="ps", bufs=6, space="PSUM") as pspool,
            tc.tile_pool(name="acc", bufs=1, space="PSUM") as accpool,
        ):
            # ---- constants + x loads (i0 first so compute starts ASAP) ----
            wcat_sb = cpool.tile([128, 1024], f32r, name="wcat_sb")
            bcat_sb = cpool.tile([128, 5], f32, name="bcat_sb")
            ident = cpool.tile([128, 128], f32, name="ident")

            loop_ctx = ExitStack()
            if loop_n > 1:
                loop_ctx.enter_context(tc.For_i(0, loop_n, 1))

            x_tiles = [xpool.tile([128, 2, HW], f32r, name=f"x_sb{b}", tag="x")
                       for b in range(BPC)]
            # batch 0, chunk i0 first; then weights; then the rest
            nc.sync.dma_start(wcat_sb[:], wcat_d[:].bitcast(f32r))
            for c in range(2):
                nc.sync.dma_start(x_tiles[0][:, c, 0:512],
                                  x_d[0, c * 128:(c + 1) * 128, 0:512].bitcast(f32r))
            nc.sync.dma_start(bcat_sb[:], bcat_d[:])
            make_identity(nc, ident[:])
            for b in range(BPC):
                for (lo, hi) in ((512, 1536), (1536, 2560), (2560, 3584), (3584, 4096)):
                    for c in range(2):
                        nc.sync.dma_start(x_tiles[b][:, c, lo:hi],
                                          x_d[b, c * 128:(c + 1) * 128, lo:hi].bitcast(f32r))
                if b > 0:
                    for c in range(2):
                        nc.sync.dma_start(x_tiles[b][:, c, 0:512],
                                          x_d[b, c * 128:(c + 1) * 128, 0:512].bitcast(f32r))

            wth_sb = wcat_sb[:, 0:256].rearrange("p (k m) -> p k m", k=2)
            wph_sb = wcat_sb[:, 256:512].rearrange("p (k m) -> p k m", k=2)
            wg_sb = wcat_sb[:, 512:768].rearrange("p (k m) -> p k m", k=2)
            wout_sb = wcat_sb[:, 768:1024]
            bth_sb = bcat_sb[:, 0:1]
            bph_sb = bcat_sb[:, 1:2]
            bg_sb = bcat_sb[:, 2:3]
            bout_sb = bcat_sb[:, 3:5]

            for b in range(BPC):
                x_sb = x_tiles[b]

                # ---- theta/phi/g convs ----
                th_sb = bigpool.tile([128, HW], f32r, name="th_sb", tag="th")
                ph_sb = bigpool.tile([128, HW], f32r, name="ph_sb", tag="ph")
                g_sb = bigpool.tile([128, HW], f32, name="g_sb", tag="g")
                # i-chunks in pairs so each weight load serves 2 matmuls
                for i2 in range(NI // 2):
                    iA, iB = 2 * i2, 2 * i2 + 1
                    slA = slice(iA * 512, (iA + 1) * 512)
                    slB = slice(iB * 512, (iB + 1) * 512)
                    for k, (w_sb, b_sb, o_sb) in enumerate((
                        (wth_sb, bth_sb, th_sb),
                        (wph_sb, bph_sb, ph_sb),
                        (wg_sb, bg_sb, g_sb),
                    )):
                        psA = pspool.tile([128, 512], f32, name="psA", tag="ps")
                        psB = pspool.tile([128, 512], f32, name="psB", tag="ps")
                        for c in range(2):
                            nc.tensor.matmul(psA[:], w_sb[:, c, :], x_sb[:, c, slA],
                                             start=(c == 0), stop=(c == 1))
                            nc.tensor.matmul(psB[:], w_sb[:, c, :], x_sb[:, c, slB],
                                             start=(c == 0), stop=(c == 1))
                        if k % 2 == 0:
                            nc.scalar.activation(o_sb[:, slA], psA[:], AF.Identity,
                                                 bias=b_sb[:], scale=1.0)
                            nc.vector.tensor_scalar_add(o_sb[:, slB], psB[:], b_sb[:])
                        else:
                            nc.vector.tensor_scalar_add(o_sb[:, slA], psA[:], b_sb[:])
                            nc.scalar.activation(o_sb[:, slB], psB[:], AF.Identity,
                                                 bias=b_sb[:], scale=1.0)

                # gT tiles produced inside the first main-loop chunk (below)
                gt_sb = bigpool.tile([128, HW], f32r, name="gt_sb", tag="gt")

                # ---- main attention loop, i-chunk pairs share stationaries ----
                y_sb = bigpool.tile([128, HW], f32r, name="y_sb", tag="y")

                def emit_outconv_half(i, o):
                    isl2 = slice(i * 512, (i + 1) * 512)
                    ps2 = pspool.tile([128, 512], f32, name="ps2", tag="ps")
                    nc.tensor.matmul(
                        ps2[:], wout_sb[:, o * 128:(o + 1) * 128], y_sb[:, isl2],
                        start=True, stop=True)
                    ot = otpool.tile([128, 512], f32, name="ot", tag="ot")
                    nc.vector.scalar_tensor_tensor(
                        ot[:], ps2[:], bout_sb[:, o:o + 1], x_sb[:, o, isl2].bitcast(f32),
                        op0=ALU.add, op1=ALU.add)
                    nc.sync.dma_start(out_d[b, o * 128:(o + 1) * 128, isl2], ot[:])

                for i2 in range(NI // 2):
                    iA, iB = 2 * i2, 2 * i2 + 1
                    slA = slice(iA * 512, (iA + 1) * 512)
                    slB = slice(iB * 512, (iB + 1) * 512)
                    accA = accpool.tile([128, 512], f32, name="accA", tag="accA")
                    accB = accpool.tile([128, 512], f32, name="accB", tag="accB")
                    pend = []
                    for j in range(NJ):
                        jsl = slice(j * 128, (j + 1) * 128)
                        psfA = pspool.tile([128, 512], f32, name="psfA", tag="ps")
                        psfB = pspool.tile([128, 512], f32, name="psfB", tag="ps")
                        # one ph_j weight load feeds both fT matmuls
                        nc.tensor.matmul(psfA[:], ph_sb[:, jsl], th_sb[:, slA],
                                         start=True, stop=True)
                        nc.tensor.matmul(psfB[:], ph_sb[:, jsl], th_sb[:, slB],
                                         start=True, stop=True)
                        ftA = ftpool.tile([128, 512], f32r, name="ftA", tag="ft")
                        ftB = ftpool.tile([128, 512], f32r, name="ftB", tag="ft")
                        if i2 == 0:
                            # produce gT_j here so the transpose phase overlaps
                            tp = pspool.tile([128, 128], f32, name="tp", tag="ps")
                            nc.tensor.transpose(tp[:], g_sb[:, jsl], ident[:])
                            if j % 2 == 0:
                                nc.scalar.copy(ftA[:], psfA[:])
                                nc.vector.tensor_copy(ftB[:], psfB[:])
                                nc.vector.tensor_copy(gt_sb[:, jsl], tp[:])
                            else:
                                nc.vector.tensor_copy(ftA[:], psfA[:])
                                nc.scalar.copy(ftB[:], psfB[:])
                                nc.scalar.copy(gt_sb[:, jsl], tp[:])
                        else:
                            if j % 2 == 0:
                                nc.scalar.copy(ftA[:], psfA[:])
                                nc.vector.tensor_copy(ftB[:], psfB[:])
                            else:
                                nc.vector.tensor_copy(ftA[:], psfA[:])
                                nc.scalar.copy(ftB[:], psfB[:])
                        pend.append((j, ftA, ftB))
                        if len(pend) > 2:
                            jj, fA, fB = pend.pop(0)
                            gsl = slice(jj * 128, (jj + 1) * 128)
                            # one gt_j weight load feeds both y matmuls
                            nc.tensor.matmul(accA[:], gt_sb[:, gsl], fA[:],
                                             start=(jj == 0), stop=(jj == NJ - 1))
                            nc.tensor.matmul(accB[:], gt_sb[:, gsl], fB[:],
                                             start=(jj == 0), stop=(jj == NJ - 1))
                        if i2 > 0:
                            # out-conv of the previous pair, spread across the j loop
                            if j == 4:
                                emit_outconv_half(2 * i2 - 2, 0)
                            elif j == 8:
                                emit_outconv_half(2 * i2 - 2, 1)
                            elif j == 12:
                                emit_outconv_half(2 * i2 - 1, 0)
                            elif j == 16:
                                emit_outconv_half(2 * i2 - 1, 1)
                    for (jj, fA, fB) in pend:
                        gsl = slice(jj * 128, (jj + 1) * 128)
                        nc.tensor.matmul(accA[:], gt_sb[:, gsl], fA[:],
                                         start=(jj == 0), stop=(jj == NJ - 1))
                        nc.tensor.matmul(accB[:], gt_sb[:, gsl], fB[:],
                                         start=(jj == 0), stop=(jj == NJ - 1))
                    nc.scalar.copy(y_sb[:, slA], accA[:])
                    nc.vector.tensor_copy(y_sb[:, slB], accB[:])
                for (i, o) in ((NI - 2, 0), (NI - 2, 1), (NI - 1, 0), (NI - 1, 1)):
                    emit_outconv_half(i, o)

            loop_ctx.close()

    nc.compile()
    return nc


def _get_program(loop_n=1):
    key = ("nc", loop_n)
    if key not in _CACHE:
        _CACHE[key] = _build_program(loop_n)
    return _CACHE[key]


def _make_in_maps(inputs):
    """Host-side prep: fold BN/bias/scale, pack weights, slice batches."""
    x = np.asarray(inputs["x"], dtype=np.float32)
    w_theta = inputs["w_theta"]; b_theta = inputs["b_theta"]
    w_phi = inputs["w_phi"]; b_phi = inputs["b_phi"]
    w_g = inputs["w_g"]; b_g = inputs["b_g"]
    w_out = inputs["w_out"]; b_out = inputs["b_out"]
    bn_gamma = inputs["bn_gamma"]; bn_beta = inputs["bn_beta"]
    bn_mean = inputs["bn_mean"]; bn_var = inputs["bn_var"]
    w_theta = np.asarray(w_theta, np.float32); b_theta = np.asarray(b_theta, np.float32)
    w_phi = np.asarray(w_phi, np.float32); b_phi = np.asarray(b_phi, np.float32)
    w_g = np.asarray(w_g, np.float32); b_g = np.asarray(b_g, np.float32)
    w_out = np.asarray(w_out, np.float32); b_out = np.asarray(b_out, np.float32)
    bn_gamma = np.asarray(bn_gamma, np.float32); bn_beta = np.asarray(bn_beta, np.float32)
    bn_mean = np.asarray(bn_mean, np.float32); bn_var = np.asarray(bn_var, np.float32)

    # host-side folding
    s = bn_gamma / np.sqrt(bn_var + BN_EPS)              # BN scale
    wout_f = (s[:, None] * w_out)                        # [C, IC]
    bout_f = s * b_out + bn_beta - bn_mean * s           # [C]

    wth = w_theta.T / HW                                 # [C, IC], 1/HW folded
    wph = w_phi.T
    wg = w_g.T
    wout = wout_f.T                                      # [IC, C]

    # wcat: [wth_c0|wth_c1|wph_c0|wph_c1|wg_c0|wg_c1|woutT] -> [128, 1024]
    wcat = np.concatenate(
        [wth[0:128], wth[128:256], wph[0:128], wph[128:256],
         wg[0:128], wg[128:256], wout], axis=1).astype(np.float32)
    wcat = np.ascontiguousarray(wcat)
    # bcat: [bth, bph, bg, bout_c0, bout_c1] -> [128, 5]
    bcat = np.stack(
        [b_theta / HW, b_phi, b_g, bout_f[0:128], bout_f[128:256]],
        axis=1).astype(np.float32)
    bcat = np.ascontiguousarray(bcat)

    xr = np.ascontiguousarray(x.reshape(B, C, HW))

    in_maps = []
    for core in range(NCORES):
        in_maps.append({
            "x": xr[core * BPC:(core + 1) * BPC],
            "wcat": wcat, "bcat": bcat,
        })
    return in_maps


def kernel(x, w_theta, b_theta, w_phi, b_phi, w_g, b_g, w_out, b_out,
           bn_gamma, bn_beta, bn_mean, bn_var):
    from concourse.bass_utils import run_bass_kernel_spmd

    in_maps = _make_in_maps(dict(
        x=x, w_theta=w_theta, b_theta=b_theta, w_phi=w_phi, b_phi=b_phi,
        w_g=w_g, b_g=b_g, w_out=w_out, b_out=b_out, bn_gamma=bn_gamma,
        bn_beta=bn_beta, bn_mean=bn_mean, bn_var=bn_var))
    nc = _get_program()
    res = run_bass_kernel_spmd(nc, in_maps, core_ids=list(range(NCORES)))
    out = np.concatenate([res.results[c]["out"] for c in range(NCORES)], axis=0)
    return out.reshape(B, C, H, W)
